# revision 42
# baseline (speedup 1.0000x reference)
"""Trainium2 Bass kernel for AdaNSABlock (7x7 neighborhood attention block).

Sharding: 8 cores = batch(4) x row-halves(2). Each core computes 16 image
rows (512 tokens) of one sample, reading 19 rows (3-row halo) of input.
Bottom halves are row-flipped on host so all cores run one SPMD graph.

v1 rewrite vs baseline:
  - input DMAs spread across sync/scalar/gpsimd queues, QKV weights first
  - LN normalize via ACT Identity (scale=rstd, bias=-mean*rstd)
  - Q/K bias folded into ACT Identity evacuation (no bias matmuls)
  - V key-windows computed directly from xhatT column slices (no SBUF DMAs)
  - attention software-pipelined at (group, quad) granularity:
      PE stream:  S(u) | pB(u-2) | AV(u-1); PSUM 4 S-banks + 4 AV-banks
      Scalar: exp;  Vector: E-mult + den/recip;  GpSimd: pB evac + normalize
  - proj on 128-row packed attnT (K=128), gelu table preloaded off-path
  - keepalive matmuls bridge PE gaps to hold the p-state ramp
"""

import numpy as np
import ml_dtypes

KS = 7
HEADS = 8
DIM = 256
HID = 1024
HD = 32
H = 32
W = 32
NT = 19 * 32          # local tokens incl halo
NQ = 512              # query tokens per core
EPS = 1e-5
BF16 = ml_dtypes.bfloat16

# token tiles covering NT
TOK_TILES = [(0, 128), (128, 128), (256, 128), (384, 128), (512, 96)]
# groups: (key_base_token, chunk_starts)
GROUPS = [(0, (0, 96)), (32, (0, 128, 192)), (160, (0, 128, 192)), (288, (0, 128, 192))]
# distinct V key-windows (start tokens, each 128 wide), in first-use order
WINDOW_STARTS = [0, 96, 32, 160, 224, 288, 352, 416, 480]
WIN_IDX = {s: i for i, s in enumerate(WINDOW_STARTS)}

MAGIC = 0x5F3759DF

_CACHE = {}


def _bf(x):
    return np.ascontiguousarray(np.asarray(x, np.float32).astype(BF16))


def _f32(x):
    return np.ascontiguousarray(np.asarray(x, np.float32))


# --------------------------------------------------------------------------
# Host-side folding + mask construction
# --------------------------------------------------------------------------

def _fold_weights(inp):
    quality = inp['quality']
    s = int(quality) - 1
    l = float(quality % 1)
    g1 = np.abs(np.asarray(inp['gamma_1'], np.float64))
    g2 = np.abs(np.asarray(inp['gamma_2'], np.float64))
    if s == g1.shape[0] - 1:
        G1, G2 = g1[s], g2[s]
    else:
        G1 = g1[s] ** (1 - l) * g1[s + 1] ** l
        G2 = g2[s] ** (1 - l) * g2[s + 1] ** l

    qkv_w = np.asarray(inp['qkv_w'], np.float64)
    qkv_b = np.asarray(inp['qkv_b'], np.float64)
    n1w = np.asarray(inp['norm1_w'], np.float64)
    n1b = np.asarray(inp['norm1_b'], np.float64)
    Wq = qkv_w * n1w[None, :]
    bq = qkv_b + qkv_w @ n1b
    sc = HD ** -0.5
    Wq[:DIM] *= sc
    bq[:DIM] *= sc

    pw = G1[:, None] * np.asarray(inp['proj_w'], np.float64)
    pb = G1 * np.asarray(inp['proj_b'], np.float64)

    n2w = np.asarray(inp['norm2_w'], np.float64)
    n2b = np.asarray(inp['norm2_b'], np.float64)
    f1w = np.asarray(inp['fc1_w'], np.float64) * n2w[None, :]
    f1b = np.asarray(inp['fc1_b'], np.float64) + np.asarray(inp['fc1_w'], np.float64) @ n2b
    f2w = G2[:, None] * np.asarray(inp['fc2_w'], np.float64)
    f2b = G2 * np.asarray(inp['fc2_b'], np.float64)

    Wv = Wq[2 * DIM:]            # [256 vdims, 256 c]
    bv = bq[2 * DIM:]
    # ones-augmented V: pair p = heads (2p, 2p+1), V'' cols 97p..97p+96:
    #   col 0..31  = dims of head 2p      (psum rows 0..31)
    #   col 32     = one_a                (psum row 32 = denom_a)
    #   col 64..95 = dims of head 2p+1    (psum rows 64..95)
    #   col 96     = one_b                (psum row 96 = denom_b)
    # head-a AV matmul: M=64 cols [0:64); head-b: M=33 cols [64:97)
    # (partition offsets 0/32/64/96 keep every slice quarter-aligned)
    Wv_aug = np.zeros((DIM, 388))
    vb_aug = np.zeros(388)
    for p in range(4):
        ha, hb = 2 * p, 2 * p + 1
        base = 97 * p
        Wv_aug[:, base:base + 32] = Wv[32 * ha:32 * ha + 32].T
        vb_aug[base:base + 32] = bv[32 * ha:32 * ha + 32]
        vb_aug[base + 32] = 1.0
        Wv_aug[:, base + 64:base + 96] = Wv[32 * hb:32 * hb + 32].T
        vb_aug[base + 64:base + 96] = bv[32 * hb:32 * hb + 32]
        vb_aug[base + 96] = 1.0

    def kblocked(wT, kb):
        # [kb*128, N] -> [128, kb, N] (partition-major, contiguous for DMA)
        n = wT.shape[1]
        return np.ascontiguousarray(wT.reshape(kb, 128, n).transpose(1, 0, 2))

    # psel2 for recip broadcast: one K=2 matmul, lhsT [2, 97]
    # (rc16 row0 = 1/den_a -> pB rows 0..63, row1 = 1/den_b -> rows 64..96)
    # smallw bf16 [2, 1094]: psel2 | vb | pb | f2b (rows beyond 0 only psel)
    smallw = np.zeros((2, 1094))
    smallw[0, 0:64] = 1.0
    smallw[1, 64:97] = 1.0
    smallw[0, 97:485] = vb_aug
    smallw[0, 485:741] = pb
    smallw[0, 741:997] = f2b

    # cbias f32 [128, 12]: qkb (q0,q1,k0,k1) | f1b (8 cols)
    cbias = np.zeros((128, 12))
    for mt in range(4):
        cbias[:, mt] = bq[128 * mt:128 * mt + 128]
    for mh in range(8):
        cbias[:, 4 + mh] = f1b[128 * mh:128 * mh + 128]

    # proj lhsT comes from attnT pair tiles [97, 128]: pair p rows 0-31 =
    # head 2p (+den rows 32/96 junk -> zero weight), rows 64-95 = head 2p+1.
    pwT_aug = np.zeros((4, 97, DIM))
    for p in range(4):
        ha, hb = 2 * p, 2 * p + 1
        pwT_aug[p, 0:32] = pw[:, 32 * ha:32 * ha + 32].T
        pwT_aug[p, 64:96] = pw[:, 32 * hb:32 * hb + 32].T

    return dict(
        wqk=_bf(kblocked(Wq[:512].T, 2)),       # [128, 2, 512]
        wv=_bf(kblocked(Wv_aug, 2)),            # [128, 2, 388]
        f1w=_bf(kblocked(f1w.T, 2)),            # [128, 2, 1024]
        f2w=_bf(kblocked(f2w.T, 8)),            # [128, 8, 256]
        pwT=_bf(np.ascontiguousarray(pwT_aug.transpose(1, 0, 2))),  # [97, 4, 256]
        smallw=_bf(smallw),                     # [2, 1094]
        cbias=_f32(cbias),                      # [128, 12]
        rpb=np.asarray(inp['rpb'], np.float64),
    )


def _build_E(rpb, flip):
    """Vectorized E (exp of bias, masked/dedup-zeroed).
    Returns E_edge [8,128,256], E_std [8,128,384] float32."""
    def img_row(r):
        return (31 - r) if flip else r

    def make(group):
        if group == 0:
            keybase, chunk_starts = 0, np.array([0, 96])
        else:
            keybase, chunk_starts = (4 * group - 3) * 32, np.array([0, 128, 192])
        nch = len(chunk_starts)
        a = np.arange(4)[:, None, None, None]         # q row in group
        qj = np.arange(32)[None, :, None, None]
        c = np.arange(nch)[None, None, :, None]
        kk = np.arange(128)[None, None, None, :]
        key = chunk_starts[c] + kk                    # rel key idx
        tloc = (keybase + key) // 32
        kj = (keybase + key) % 32
        rloc_q = 4 * group + a
        qi = img_row(rloc_q)
        ki = img_row(tloc)
        sh = np.clip(qi - 3, 0, H - KS)
        sw = np.clip(qj - 3, 0, H - KS)
        valid = (ki >= sh) & (ki < sh + KS) & (kj >= sw) & (kj < sw + KS)
        if nch > 1:
            dedup = ~((c > 0) & (key < chunk_starts[np.maximum(c - 1, 0)] + 128))
            valid = valid & dedup
        bh = np.clip(ki - qi + KS - 1, 0, 2 * KS - 2)
        bw = np.clip(kj - qj + KS - 1, 0, 2 * KS - 2)
        # [8, 4, 32, nch, 128]
        bias = rpb[:, bh, bw]
        E = np.where(valid[None], np.exp(bias), 0.0)
        return np.ascontiguousarray(
            E.reshape(HEADS, 4 * 32, nch * 128).astype(np.float32))
    return make(0), make(1)


def _prepare_inputs(inp):
    F = _fold_weights(inp)
    E_e_t, E_s_t = _build_E(F['rpb'], flip=False)
    E_e_b, E_s_b = _build_E(F['rpb'], flip=True)
    x = np.asarray(inp['x'], np.float32)
    Bsz = x.shape[0]
    shared = {k: v for k, v in F.items() if k != 'rpb'}
    in_maps = []
    for b in range(Bsz):
        for half in range(2):
            if half == 0:
                x_loc = x[b, 0:19].reshape(NT, DIM)
                Ee, Es = E_e_t, E_s_t
            else:
                x_loc = x[b, 31:12:-1].reshape(NT, DIM)
                Ee, Es = E_e_b, E_s_b
            m = dict(shared)
            m['x'] = _f32(x_loc)
            # device layout: [key-within-chunk(128), head, chunk*128 + q]
            def dev(E):
                nch = E.shape[2] // 128
                return np.ascontiguousarray(
                    E.reshape(HEADS, 128, nch, 128)
                    .transpose(3, 0, 2, 1)
                    .reshape(128, HEADS * nch * 128))
            m['Eall'] = _bf(np.concatenate([dev(Ee), dev(Es)], axis=1))
            in_maps.append(m)
    return in_maps


# --------------------------------------------------------------------------
# Bass kernel graph
# --------------------------------------------------------------------------

def build_graph():
    import concourse.bass as bass
    import concourse.tile as tile
    import concourse.mybir as mybir
    from concourse import bacc
    from concourse.masks import make_identity

    dt = mybir.dt
    Alu = mybir.AluOpType
    Act = mybir.ActivationFunctionType

    nc = bacc.Bacc()

    def param(name, shape, dtype, out=False):
        return nc.declare_dram_parameter(name, list(shape), dtype, isOutput=out)

    x_d = param("x", (NT, DIM), dt.float32)
    wqk_d = param("wqk", (128, 2, 512), dt.bfloat16)
    wv_d = param("wv", (128, 2, 388), dt.bfloat16)
    f1w_d = param("f1w", (128, 2, HID), dt.bfloat16)
    f2w_d = param("f2w", (128, 8, DIM), dt.bfloat16)
    pwT_d = param("pwT", (97, 4, DIM), dt.bfloat16)
    smallw_d = param("smallw", (2, 1094), dt.bfloat16)
    cbias_d = param("cbias", (128, 12), dt.float32)
    Eall_d = param("Eall", (128, HEADS * (256 + 384)), dt.bfloat16)
    out_d = param("out", (NQ, DIM), dt.float32, out=True)

    with tile.TileContext(nc) as tc:
        with (
            tc.tile_pool(name="consts", bufs=1) as consts,
            tc.tile_pool(name="persist", bufs=1) as persist,
            tc.tile_pool(name="work", bufs=3) as work,
            tc.tile_pool(name="aq", bufs=3) as aqpool,
            tc.tile_pool(name="ps", bufs=6, space="PSUM") as ps,
            tc.tile_pool(name="psav", bufs=2, space="PSUM") as psav,
        ):
            # ---------------- input DMAs ---------------------------------
            # All issued from sync in need-order: transfers appear to drain
            # a shared DMA device serially, so order = landing order.
            x_tiles = []
            for t, (off, nt) in enumerate(TOK_TILES):
                xt = persist.tile([128, DIM], dt.float32, tag=f"x{t}", name=f"x{t}")
                nc.sync.dma_start(out=xt[:nt], in_=x_d[off:off + nt, :])
                x_tiles.append(xt)
            cbias_sb = consts.tile([128, 12], dt.float32, tag="cbias")
            nc.sync.dma_start(out=cbias_sb, in_=cbias_d[:])
            wqk_sb = consts.tile([128, 2, 512], dt.bfloat16, tag="wqk")
            nc.sync.dma_start(out=wqk_sb, in_=wqk_d[:])
            smallw_sb = consts.tile([2, 1094], dt.bfloat16, tag="smallw")
            nc.sync.dma_start(out=smallw_sb, in_=smallw_d[:])
            wv_sb = consts.tile([128, 2, 388], dt.bfloat16, tag="wv")
            nc.sync.dma_start(out=wv_sb, in_=wv_d[:])
            Eall_sb = consts.tile([128, HEADS * 640], dt.bfloat16, tag="Eall")
            nc.sync.dma_start(out=Eall_sb, in_=Eall_d[:])
            pwT_sb = consts.tile([97, 4, DIM], dt.bfloat16, tag="pwT")
            nc.sync.dma_start(out=pwT_sb, in_=pwT_d[:])
            f1w_sb = consts.tile([128, 2, HID], dt.bfloat16, tag="f1w")
            nc.sync.dma_start(out=f1w_sb, in_=f1w_d[:])
            f2w_sb = consts.tile([128, 8, DIM], dt.bfloat16, tag="f2w")
            nc.sync.dma_start(out=f2w_sb, in_=f2w_d[:])

            psel_sb = smallw_sb[0:2, 0:97]
            vb_sb = smallw_sb[0:1, 97:485]
            pb_sb = smallw_sb[0:1, 485:741]
            f2b_sb = smallw_sb[0:1, 741:997]
            qkb_sb = cbias_sb[:, 0:4]
            f1b_sb = cbias_sb[:, 4:12]

            ident = consts.tile([128, 128], dt.bfloat16, tag="ident")
            make_identity(nc, ident)
            Ee_sb = Eall_sb[:, 0:HEADS * 256].rearrange("p (h c) -> p h c", h=HEADS)
            Es_sb = Eall_sb[:, HEADS * 256:].rearrange("p (h c) -> p h c", h=HEADS)

            ones_sb = consts.tile([1, NT], dt.bfloat16, tag="ones")
            nc.vector.memset(ones_sb, 1.0)

            # ---------------- PE keepalive (p-state ramp) -----------------
            def keepalive(n, nk=128):
                ka = ps.tile([128, 512], dt.float32, tag="pss", name="ka")
                for _ in range(n):
                    nc.tensor.matmul(ka[:, :nk], ident, ident[:, :nk],
                                     start=True, stop=True)

            keepalive(75)

            # ---------------- helpers ----------------
            def dve_rsqrt(dst, src, n):
                """dst[:,0:n] = 1/sqrt(src[:,0:n] + EPS); small-n f32 tiles."""
                ve = work.tile([128, n], dt.float32, tag="rsq_ve", bufs=2)
                nc.vector.tensor_scalar(out=ve, in0=src, scalar1=float(EPS),
                                        scalar2=None, op0=Alu.add)
                yi = work.tile([128, n], dt.int32, tag="rsq_yi", bufs=2)
                nc.vector.tensor_scalar(out=yi, in0=ve[:].bitcast(dt.int32),
                                        scalar1=1, scalar2=None,
                                        op0=Alu.logical_shift_right)
                nc.vector.tensor_scalar(out=yi, in0=yi, scalar1=-1,
                                        scalar2=MAGIC, op0=Alu.mult, op1=Alu.add)
                y = yi[:].bitcast(dt.float32)
                t = work.tile([128, n], dt.float32, tag="rsq_t", bufs=2)
                for _ in range(2):
                    nc.vector.tensor_tensor(out=t, in0=y, in1=y, op=Alu.mult)
                    nc.vector.tensor_tensor(out=t, in0=t, in1=ve, op=Alu.mult)
                    nc.vector.tensor_scalar(out=t, in0=t, scalar1=-0.5,
                                            scalar2=1.5, op0=Alu.mult, op1=Alu.add)
                    nc.vector.tensor_tensor(out=y, in0=y, in1=t, op=Alu.mult)
                nc.vector.tensor_copy(out=dst, in_=y)

            def ln_batch(x_list, sizes, tagp):
                """LayerNorm a batch of tiles. Vector: stats + 1/(v+eps);
                Scalar: sqrt -> rstd; Vector: -m*rstd;
                Scalar: per-tile Identity ACT (x*r - m*r) -> bf16 xhat.
                Returns list of (xhat_tile, nt)."""
                ntile = len(x_list)
                mv = work.tile([128, 2 * ntile], dt.float32, tag=tagp + "_mv", bufs=2)
                nc.vector.memset(mv, 1.0)
                for t, (xt, nt) in enumerate(zip(x_list, sizes)):
                    stats = work.tile([128, 6], dt.float32, tag=tagp + "_st", bufs=2)
                    nc.vector.bn_stats(out=stats[:nt], in_=xt[:nt])
                    nc.vector.bn_aggr(out=mv[:nt, 2 * t:2 * t + 2], in_=stats[:nt])
                rstd = work.tile([128, ntile], dt.float32, tag=tagp + "_rs", bufs=2)
                dve_rsqrt(rstd, mv[:, 1::2], ntile)
                negmr = work.tile([128, ntile], dt.float32, tag=tagp + "_nm", bufs=2)
                nc.vector.scalar_tensor_tensor(out=negmr, in0=mv[:, 0::2],
                                               scalar=-1.0, in1=rstd,
                                               op0=Alu.mult, op1=Alu.mult)
                outs = []
                for t, (xt, nt) in enumerate(zip(x_list, sizes)):
                    xh = persist.tile([128, DIM], dt.bfloat16,
                                      tag=f"{tagp}_xh{t}", name=f"{tagp}_xh{t}")
                    nc.scalar.activation(out=xh[:nt], in_=xt[:nt],
                                         func=Act.Identity,
                                         scale=rstd[:nt, t:t + 1],
                                         bias=negmr[:nt, t:t + 1])
                    outs.append((xh, nt))
                return outs

            def make_tcat(tagp, total):
                return [persist.tile([128, total], dt.bfloat16,
                                     tag=f"{tagp}_{cb}", name=f"{tagp}_{cb}")
                        for cb in range(2)]

            def transpose_into(res, xh_tiles, off, evac_engines, ei0=0):
                """Transpose [nt, 256] bf16 tiles into res c-block tiles at
                column offset off. Returns next offset."""
                ei = ei0
                for xh, nt in xh_tiles:
                    for cb in range(2):
                        ptb = ps.tile([128, 128], dt.bfloat16, tag="pss", name="ptb")
                        nc.tensor.transpose(ptb[:, :nt], xh[:nt, 128 * cb:128 * (cb + 1)],
                                            ident[:nt, :nt])
                        eng = evac_engines[ei % len(evac_engines)]
                        ei += 1
                        eng.tensor_copy(out=res[cb][:, off:off + nt], in_=ptb[:, :nt])
                    off += nt
                return off

            # ---------------- LN1 (two batches) + transposes --------------
            xhatT = make_tcat("xhatT", NT)
            b1 = ln_batch(x_tiles[:3], [128, 128, 128], "ln1a")
            b2 = ln_batch(x_tiles[3:], [128, 96], "ln1b")
            transpose_into(xhatT, b1 + b2, 0, [nc.vector])

            # ---------------- QKV ----------------
            # Q (mt 0,1), K (mt 2,3): psum accum over 2 k-blocks,
            # bias via Identity-ACT evacuation.
            qT, kT = [], []
            for mt in range(4):
                ncols = NQ if mt < 2 else NT
                dst = persist.tile([128, ncols], dt.bfloat16, tag=f"qk{mt}")
                for n0 in range(0, ncols, 512):
                    nn = min(512, ncols - n0)
                    pt = ps.tile([128, 512], dt.float32, tag="pss", name="pqk")
                    for kb in range(2):
                        nc.tensor.matmul(
                            pt[:, :nn], wqk_sb[:, kb, 128 * mt:128 * (mt + 1)],
                            xhatT[kb][:, n0:n0 + nn],
                            start=(kb == 0), stop=(kb == 1))
                    # Q evacs on Scalar, K evacs on Vector (both Identity+bias
                    # capable? DVE copy can't add bias -> K bias via Scalar too)
                    nc.scalar.activation(out=dst[:, n0:n0 + nn], in_=pt[:, :nn],
                                         func=Act.Identity,
                                         bias=qkb_sb[:, mt:mt + 1])
                (qT if mt < 2 else kT).append(dst)

            # V key-windows computed lazily: scheduled into the attention
            # pipeline (see VW_SCHED) so early units start sooner.
            Vw = [None] * len(WINDOW_STARTS)

            def make_vwindow(wi):
                ws = WINDOW_STARTS[wi]
                vt = persist.tile([128, 388], dt.bfloat16, tag=f"vw{wi}", name=f"vw{wi}")
                pt = ps.tile([128, 512], dt.float32, tag="pss", name=f"pv{wi}")
                pv = pt[:, :388]
                for kb in range(2):
                    nc.tensor.matmul(pv, xhatT[kb][:, ws:ws + 128], wv_sb[:, kb, :],
                                     start=(kb == 0), stop=False)
                nc.tensor.matmul(pv, ones_sb[:, :128], vb_sb, start=False, stop=True)
                if wi % 2 == 0:
                    nc.vector.tensor_copy(out=vt, in_=pv)
                else:
                    nc.scalar.activation(out=vt, in_=pv, func=Act.Copy)
                Vw[wi] = vt

            # ---------------- attention: 8 (group, quad) units ------------
            # attnP2[quad]: [97, pair-local(2), NQ] — normalized attention^T
            attnP2 = [persist.tile([97, 2, NQ], dt.bfloat16, tag=f"attnP{q}",
                                   name=f"attnP{q}") for q in range(2)]
            # f32 pair-selector rows for the broadcast matmuls
            pself = consts.tile([1, 194], dt.float32, tag="pself")
            nc.vector.memset(pself, 0.0)
            nc.vector.memset(pself[:, 0:64], 1.0)
            nc.vector.memset(pself[:, 161:194], 1.0)

            UNITS = [(g, q) for g in range(4) for q in range(2)]

            def unit_S(u):
                """S^T matmuls + exp for unit u. Returns aq tile."""
                g, quad = UNITS[u]
                kb_tok, css = GROUPS[g]
                nch = len(css)
                aq = aqpool.tile([128, 4, 384], dt.bfloat16, tag="aq",
                                 name=f"aq_{u}")
                for slot in range(4):
                    pS = ps.tile([128, 512], dt.float32, tag="pss",
                                 name=f"pS_{u}_{slot}")
                    for c, cs in enumerate(css):
                        nc.tensor.matmul(
                            pS[:, 128 * c:128 * (c + 1)],
                            kT[quad][32 * slot:32 * slot + 32,
                                     kb_tok + cs:kb_tok + cs + 128],
                            qT[quad][32 * slot:32 * slot + 32,
                                     128 * g:128 * (g + 1)],
                            start=True, stop=True,
                            tile_position=(32 * slot, 0))
                    nc.scalar.activation(
                        out=aq[:, slot, :128 * nch],
                        in_=pS[:, :128 * nch], func=Act.Exp)
                return aq

            def unit_emult(u, aq):
                """aq *= E for unit u (pair 0 on GpSimd, pair 1 on Vector)."""
                g, quad = UNITS[u]
                nch = len(GROUPS[g][1])
                E_sb = Ee_sb if g == 0 else Es_sb
                for pl, eng in ((0, nc.gpsimd), (1, nc.vector)):
                    eng.tensor_tensor(
                        out=aq[:, 2 * pl:2 * pl + 2, :128 * nch],
                        in0=aq[:, 2 * pl:2 * pl + 2, :128 * nch],
                        in1=E_sb[:, 4 * quad + 2 * pl:4 * quad + 2 * pl + 2,
                                 :128 * nch],
                        op=Alu.mult)

            def unit_AV(u, aq):
                """AV matmuls for unit u — whole unit in ONE psum bank:
                pair0 numerators cols 0:128, pair1 cols 128:256,
                broadcasts go to 256:384 / 384:512 later.
                Returns (pav, rcd)."""
                g, quad = UNITS[u]
                kb_tok, css = GROUPS[g]
                nch = len(css)
                pav = psav.tile([128, 512], dt.float32, tag="psav",
                                name=f"pav_{u}")
                for pl in range(2):
                    p = 2 * quad + pl
                    pnum = pav[:97, 128 * pl:128 * (pl + 1)]
                    for hh_loc, po, mm in ((2 * pl, 0, 64), (2 * pl + 1, 64, 33)):
                        voff = 97 * p + (0 if po == 0 else 64)
                        for c, cs in enumerate(css):
                            vt = Vw[WIN_IDX[kb_tok + cs]]
                            nc.tensor.matmul(
                                pnum[po:po + mm, :],
                                vt[:, voff:voff + mm],
                                aq[:, hh_loc, 128 * c:128 * (c + 1)],
                                start=(c == 0), stop=(c == nch - 1),
                                tile_position=(0, po))
                # Vector: den gather (both pairs at once), one reciprocal.
                # den layout: [a0 | a1 | b0 | b1] blocks of 128
                den = work.tile([1, 512], dt.float32, tag="den", bufs=3)
                nc.vector.tensor_copy(out=den[:, 0:256], in_=pav[32:33, 0:256])
                nc.vector.tensor_copy(out=den[:, 256:512], in_=pav[96:97, 0:256])
                rcd = work.tile([1, 512], dt.float32, tag="rcd", bufs=3)
                nc.vector.reciprocal_approx_fast(out=rcd, in_=den)
                return (pav, rcd)

            def unit_pB(avout, u):
                """PE broadcast of reciprocals (f32 matmuls), one evacuation,
                one Vector normalize into attnP2."""
                g, quad = UNITS[u]
                pav, rcd = avout
                for pl in range(2):
                    pB = pav[:97, 256 + 128 * pl:384 + 128 * pl]
                    nc.tensor.matmul(pB, pself[:, 0:97],
                                     rcd[:, 128 * pl:128 * (pl + 1)],
                                     start=True, stop=False)
                    nc.tensor.matmul(pB, pself[:, 97:194],
                                     rcd[:, 256 + 128 * pl:384 + 128 * pl],
                                     start=False, stop=True)
                rcb = work.tile([97, 256], dt.bfloat16, tag="rcb", bufs=3)
                nc.vector.tensor_copy(out=rcb, in_=pav[:97, 256:512])
                nc.vector.tensor_tensor(
                    out=attnP2[quad][:, :, 128 * g:128 * (g + 1)],
                    in0=pav[:97, 0:256].rearrange("p (two q) -> p two q", two=2),
                    in1=rcb.rearrange("p (two q) -> p two q", two=2),
                    op=Alu.mult)

            # software pipeline: PE order  Vw | S(u) | pB(u-2) | AV(u-1)
            # V windows land just before the units that need them, filling
            # the early-pipeline PE bubbles.
            VW_SCHED = {0: [0, 1], 1: [2, 3], 2: [4, 5], 3: [6, 7], 4: [8]}
            aqs = [None] * 8
            avouts = [None] * 8
            for u in range(8):
                for wi in VW_SCHED.get(u, []):
                    make_vwindow(wi)
                aqs[u] = unit_S(u)
                if u >= 2:
                    unit_pB(avouts[u - 2], u - 2)
                unit_emult(u, aqs[u])
                if u >= 1:
                    avouts[u - 1] = unit_AV(u - 1, aqs[u - 1])
            avouts[7] = unit_AV(7, aqs[7])
            unit_pB(avouts[6], 6)
            unit_pB(avouts[7], 7)

            # ---------------- proj + residual ----------------
            keepalive(8)
            y_tiles = []
            for mt in range(4):
                pt = ps.tile([128, 512], dt.float32, tag="pss", name=f"pproj{mt}")
                pp = pt[:, :DIM]
                for p in range(4):
                    nc.tensor.matmul(pp,
                                     attnP2[p // 2][:, p % 2, 128 * mt:128 * (mt + 1)],
                                     pwT_sb[:, p, :], start=(p == 0), stop=False)
                nc.tensor.matmul(pp, ones_sb[:, :128], pb_sb, start=False, stop=True)
                yt = persist.tile([128, DIM], dt.float32, tag=f"y{mt}")
                nc.vector.tensor_tensor(out=yt, in0=pp, in1=x_tiles[mt][:],
                                        op=Alu.add)
                y_tiles.append(yt)

            # gelu table preload (dummy) while LN2 runs
            gdummy = work.tile([1, 2], dt.float32, tag="gdummy")
            nc.scalar.activation(out=gdummy, in_=ones_sb[0:1, 0:2], func=Act.Gelu)

            # ---------------- LN2 + MLP ----------------
            x2T = make_tcat("x2T", NQ)
            xh2a = ln_batch(y_tiles[:2], [128, 128], "ln2a")
            xh2b = ln_batch(y_tiles[2:], [128, 128], "ln2b")
            keepalive(16)
            transpose_into(x2T, xh2a + xh2b, 0, [nc.vector])

            keepalive(6)
            m1 = []
            for mh in range(8):
                pt = ps.tile([128, 512], dt.float32, tag="pss", name=f"pfc1{mh}")
                for kb in range(2):
                    nc.tensor.matmul(pt, f1w_sb[:, kb, 128 * mh:128 * (mh + 1)],
                                     x2T[kb], start=(kb == 0), stop=(kb == 1))
                mg = persist.tile([128, NQ], dt.bfloat16, tag=f"m1_{mh}")
                nc.scalar.activation(out=mg, in_=pt, func=Act.Gelu,
                                     bias=f1b_sb[:, mh:mh + 1], scale=1.0)
                m1.append(mg)

            # fc2 mt-major: finish each output tile early, DMA out alternating
            for mt in range(4):
                pt = ps.tile([128, 512], dt.float32, tag="pss", name=f"pfc2{mt}")
                pp = pt[:, :DIM]
                for kb in range(8):
                    nc.tensor.matmul(pp, m1[kb][:, 128 * mt:128 * (mt + 1)],
                                     f2w_sb[:, kb, :], start=(kb == 0), stop=False)
                nc.tensor.matmul(pp, ones_sb[:, :128], f2b_sb, start=False, stop=True)
                ot = work.tile([128, DIM], dt.float32, tag="outt", bufs=2)
                nc.vector.tensor_tensor(out=ot, in0=pp, in1=y_tiles[mt][:],
                                        op=Alu.add)
                eng = nc.sync if mt % 2 == 0 else nc.gpsimd
                eng.dma_start(out=out_d[128 * mt:128 * (mt + 1), :], in_=ot)

    nc.finalize()
    return nc


# --------------------------------------------------------------------------
# Entry point
# --------------------------------------------------------------------------

def kernel(**inputs):
    from concourse.bass_utils import run_bass_kernel_spmd

    if 'nc' not in _CACHE:
        _CACHE['nc'] = build_graph()
    nc = _CACHE['nc']

    in_maps = _prepare_inputs(inputs)
    res = run_bass_kernel_spmd(nc, in_maps, core_ids=list(range(8)))
    x = np.asarray(inputs['x'])
    Bsz, Hh, Ww, C = x.shape
    out = np.zeros((Bsz, Hh, Ww, C), np.float32)
    for i in range(2 * Bsz):
        b, half = divmod(i, 2)
        o = np.asarray(res.results[i]['out']).reshape(16, Ww, C)
        if half == 0:
            out[b, 0:16] = o
        else:
            out[b, 16:32] = o[::-1]
    return out.astype(x.dtype)
